# revision 11
# baseline (speedup 1.0000x reference)
"""EuclideanVisitEncoder forward: masked-mean embedding bag on 8 NeuronCores.

out[b, :] = sum_l (ids[b,l] != 0) * T[ids[b,l], :] / max(count_b, 1)

Sharding: data-parallel over the batch across 8 cores (25088 padded rows
each); the 6.4 MB table is replicated.

Gather strategy: the SWDGE ucode dma_gather (mlp library) moves 256-byte
elements per int16 index. The f32 table [100000, 16] is viewed as
[25000, 64] "super-rows" (4 consecutive embedding rows = 256 B); index
id>>2 fits int16. Each 128-row tile issues 4 dma_gather chunks of 2048
indices (the SWDGE descriptor ring is sized 4096 descs via
dynamic_dma_scratch_size=64KB so desc-gen overlaps the DMA drain),
landing g4[p, l, 64] = super-row of ids[p, l]. The wanted 16 floats are
selected by weights w[p, l, r] = (ids&3 == r) * (ids != 0), fused with
the mean reduction on DVE:
    out[p, d] = (sum_{l,r} w[p,l,r] * g4[p, l*64+r*16+d]) / max(cnt, 1)

Index tiles (wrapped [16 x num_idxs/16] layout, replicated to 128
partitions) are precomputed host-side from code_ids (pure re-layout of
the input), streamed per tile as an extra int16 input.

Per tile (cost model): Pool 4x(994+2048*0.34) ~= 6.8 us desc-gen, DMA
8192 descs * 256 B ~= 12 us, DVE ~10 us -> DMA-bound ~2.5 ms/core for
196 tiles (vs ~17-28 ms for the 64-instruction-per-tile indirect1d
baseline, which paid 994 ns fixed SWDGE cost per 128 gathered rows).
"""

import numpy as np

PAD_IDX = 0
NUM_CODES = 100000
DIM = 16
B, L = 200000, 64
N_CORES = 8

TILE_ROWS = 128
B_SHARD = 25088                       # 25000 + pad to multiple of 128
N_TILES = B_SHARD // TILE_ROWS        # 196

NSUP = NUM_CODES // 4                 # 25000 super-rows
ELEM = 64                             # f32 per super-row (256 B)
CHUNK = 1024                          # idxs per dma_gather (SWDGE ring cap)
NCHUNK = (TILE_ROWS * L) // CHUNK     # 8
UPT = CHUNK // TILE_ROWS              # 8 l-slots per chunk
GIDX_COLS = NCHUNK * (CHUNK // 16)    # 512 int16 per partition-row per tile
N_QUEUES = 4                          # SWDGE queues to spread chunks over

_PROGRAM_CACHE = {}


def build_program(b_shard=B_SHARD, repeats=1, unroll=14, bufs=2):
    from contextlib import ExitStack

    import concourse.tile as tile
    from concourse import bacc, bass, mybir

    n_tiles = b_shard // TILE_ROWS
    assert b_shard % TILE_ROWS == 0 and n_tiles % unroll == 0

    nc = bacc.Bacc(
        "TRN2",
        target_bir_lowering=False,
        debug=False,
        num_swdge_queues=N_QUEUES,
    )
    ids_t = nc.dram_tensor("code_ids", [b_shard, L], mybir.dt.int32, kind="ExternalInput")
    gidx_t = nc.dram_tensor("gidx", [b_shard, GIDX_COLS], mybir.dt.int16, kind="ExternalInput")
    tbl_t = nc.dram_tensor("emb4", [NSUP, ELEM], mybir.dt.float32, kind="ExternalInput")
    iota_t = nc.dram_tensor("iota4", [128, L * 4], mybir.dt.int32, kind="ExternalInput")
    out_t = nc.dram_tensor("out", [b_shard, DIM], mybir.dt.float32, kind="ExternalOutput")

    with ExitStack() as ctx:
        tc = ctx.enter_context(tile.TileContext(nc))
        const_pool = ctx.enter_context(tc.tile_pool(name="const", bufs=1))
        ids_pool = ctx.enter_context(tc.tile_pool(name="ids", bufs=bufs))
        g_pool = ctx.enter_context(tc.tile_pool(name="g", bufs=bufs))
        s_pool = ctx.enter_context(tc.tile_pool(name="s", bufs=bufs))

        iota4 = const_pool.tile([128, L * 4], mybir.dt.int32, tag="iota4")
        nc.sync.dma_start(out=iota4[:], in_=iota_t[:, :])

        def tile_body(row0_expr):
            ids_tile = ids_pool.tile([128, L], mybir.dt.int32, tag="ids", name=f"ids{nc.next_id()}")
            nc.sync.dma_start(out=ids_tile[:], in_=ids_t[bass.ds(row0_expr, 128), :])
            gidx_tile = ids_pool.tile([128, GIDX_COLS], mybir.dt.int16, tag="gidx", name=f"gi{nc.next_id()}")
            nc.sync.dma_start(out=gidx_tile[:], in_=gidx_t[bass.ds(row0_expr, 128), :])

            g4 = g_pool.tile([128, L * ELEM], mybir.dt.float32, tag="g4", name=f"g{nc.next_id()}")
            for c in range(NCHUNK):
                nc.gpsimd.dma_gather(
                    out_ap=g4[:, c * UPT * ELEM : (c + 1) * UPT * ELEM].rearrange(
                        "p (u e) -> p u e", u=UPT, e=ELEM
                    ),
                    in_ap=tbl_t[:, :],
                    idxs_ap=gidx_tile[:, c * (CHUNK // 16) : (c + 1) * (CHUNK // 16)],
                    num_idxs=CHUNK,
                    num_idxs_reg=CHUNK,
                    elem_size=ELEM,
                    queue_num=c % N_QUEUES,
                )

            # weights: w[p, l, r] = (ids&3 == r); pad rows excluded via m
            a = s_pool.tile([128, L], mybir.dt.int32, tag="a", name=f"a{nc.next_id()}")
            nc.vector.tensor_scalar(a[:], ids_tile[:], 3, None, op0=mybir.AluOpType.bitwise_and)
            m = s_pool.tile([128, L], mybir.dt.float32, tag="m", name=f"m{nc.next_id()}")
            nc.vector.tensor_scalar(m[:], ids_tile[:], 0, None, op0=mybir.AluOpType.not_equal)
            den = s_pool.tile([128, 1], mybir.dt.float32, tag="den", name=f"d{nc.next_id()}")
            nc.vector.tensor_reduce(den[:], m[:], axis=mybir.AxisListType.X, op=mybir.AluOpType.add)
            nc.vector.tensor_scalar_max(den[:], den[:], 1.0)
            recip = s_pool.tile([128, 1], mybir.dt.float32, tag="recip", name=f"r{nc.next_id()}")
            nc.vector.reciprocal(recip[:], den[:])

            wcmp = s_pool.tile([128, L * 4], mybir.dt.float32, tag="wcmp", name=f"w{nc.next_id()}")
            nc.vector.tensor_tensor(
                out=wcmp[:].rearrange("p (l r) -> p l r", l=L, r=4),
                in0=a[:].broadcast_to([128, L, 4]),
                in1=iota4[:].rearrange("p (l r) -> p l r", l=L, r=4),
                op=mybir.AluOpType.is_equal,
            )
            wm = s_pool.tile([128, L * 4], mybir.dt.float32, tag="wm", name=f"wm{nc.next_id()}")
            nc.vector.tensor_tensor(
                out=wm[:].rearrange("p (l r) -> p l r", l=L, r=4),
                in0=wcmp[:].rearrange("p (l r) -> p l r", l=L, r=4),
                in1=m[:].broadcast_to([128, L, 4]),
                op=mybir.AluOpType.mult,
            )

            P = g_pool.tile([128, L * ELEM], mybir.dt.float32, tag="P", name=f"P{nc.next_id()}")
            nc.vector.tensor_tensor(
                out=P[:].rearrange("p (l r d) -> p l r d", l=L, r=4, d=DIM),
                in0=g4[:].rearrange("p (l r d) -> p l r d", l=L, r=4, d=DIM),
                in1=wm[:].rearrange("p (l r) -> p l r", l=L, r=4).broadcast_to([128, L, 4, DIM]),
                op=mybir.AluOpType.mult,
            )
            acc = s_pool.tile([128, DIM], mybir.dt.float32, tag="acc", name=f"ac{nc.next_id()}")
            nc.vector.tensor_reduce(
                acc[:],
                P[:].rearrange("p (lr d) -> p d lr", lr=L * 4, d=DIM),
                axis=mybir.AxisListType.X,
                op=mybir.AluOpType.add,
            )
            outt = s_pool.tile([128, DIM], mybir.dt.float32, tag="outt", name=f"o{nc.next_id()}")
            nc.vector.tensor_scalar(outt[:], acc[:], recip[:], None, op0=mybir.AluOpType.mult)
            nc.sync.dma_start(out=out_t[bass.ds(row0_expr, 128), :], in_=outt[:])

        if repeats == 1:
            with tc.For_i(0, n_tiles // unroll, 1) as i:
                for u in range(unroll):
                    tile_body(i * (128 * unroll) + u * 128)
        else:
            with tc.For_i(0, repeats, 1) as _r:
                with tc.For_i(0, n_tiles // unroll, 1) as i:
                    for u in range(unroll):
                        tile_body(i * (128 * unroll) + u * 128)

    nc.compile()
    return nc


def _get_program():
    key = (B_SHARD, 1)
    if key not in _PROGRAM_CACHE:
        _PROGRAM_CACHE[key] = build_program()
    return _PROGRAM_CACHE[key]


def _make_gidx(ids_shard: np.ndarray) -> np.ndarray:
    """Wrapped dma_gather index layout for one core shard [b_shard, L].

    Tile t, chunk c, in-chunk j = u*128+p gathers super-row of
    ids[t*128+p, c*UPT+u]; idx j lives at partition j%16 (replicated to
    all 8 16-partition groups), column c*(CHUNK/16) + j//16 of the
    [128, L] int16 per-tile block.
    """
    b_shard = ids_shard.shape[0]
    n_tiles = b_shard // TILE_ROWS
    sup = (ids_shard >> 2).astype(np.int16)          # [b, L]
    s = sup.reshape(n_tiles, TILE_ROWS, NCHUNK, UPT)  # [t, p, c, u]
    s = s.transpose(0, 2, 3, 1)                       # [t, c, u, p]
    s = s.reshape(n_tiles, NCHUNK, CHUNK // 16, 16)   # [t, c, j//16, j%16]
    s = s.transpose(0, 1, 3, 2)                       # [t, c, q, col]
    s = np.tile(s, (1, 1, 8, 1))                      # [t, c, 128, col]
    s = s.transpose(0, 2, 1, 3)                       # [t, 128, c, col]
    return np.ascontiguousarray(s.reshape(b_shard, GIDX_COLS))


def make_in_maps(code_ids: np.ndarray, emb_weight: np.ndarray):
    code_ids = np.ascontiguousarray(np.asarray(code_ids), dtype=np.int32)
    emb_weight = np.ascontiguousarray(np.asarray(emb_weight), dtype=np.float32)
    tbl4 = emb_weight.reshape(NSUP, ELEM)
    b_total = N_CORES * B_SHARD
    ids_pad = np.zeros((b_total, L), dtype=np.int32)
    ids_pad[: code_ids.shape[0], :] = code_ids
    iota4 = np.tile(np.arange(4, dtype=np.int32), (128, L)).reshape(128, L * 4)
    maps = []
    for i in range(N_CORES):
        shard = ids_pad[i * B_SHARD : (i + 1) * B_SHARD]
        maps.append(
            {
                "code_ids": shard,
                "gidx": _make_gidx(shard),
                "emb4": tbl4,
                "iota4": iota4,
            }
        )
    return maps


def kernel(code_ids: np.ndarray, emb_weight: np.ndarray, **kwargs) -> np.ndarray:
    from concourse import bass_utils

    nc = _get_program()
    in_maps = make_in_maps(code_ids, emb_weight)
    res = bass_utils.run_bass_kernel_spmd(nc, in_maps, core_ids=list(range(N_CORES)))
    out = np.concatenate([res.results[i]["out"] for i in range(N_CORES)], axis=0)
    return out[: np.asarray(code_ids).shape[0]]


if __name__ == "__main__":
    rng = np.random.default_rng(0)
    ids = rng.integers(0, NUM_CODES, size=(B, L)).astype(np.int32)
    w = rng.standard_normal((NUM_CODES, DIM)).astype(np.float32)
    o = kernel(code_ids=ids, emb_weight=w)
    print(o.shape, o.dtype, o[:2, :4])


# revision 12
# speedup vs baseline: 21.0241x; 21.0241x over previous
"""EuclideanVisitEncoder forward: masked-mean embedding bag on 8 NeuronCores.

out[b, :] = sum_l (ids[b,l] != 0) * T[ids[b,l], :] / max(count_b, 1)

Sharding: data-parallel over the batch across 8 cores (25088 padded rows
each); the 6.4 MB table is replicated.

Gather strategy: the SWDGE ucode dma_gather (mlp library) moves 256-byte
elements per int16 index. The f32 table [100000, 16] is viewed as
[25000, 64] "super-rows" (4 consecutive embedding rows = 256 B); index
id>>2 fits int16. Each 128-row tile issues 4 dma_gather chunks of 2048
indices (the SWDGE descriptor ring is sized 4096 descs via
dynamic_dma_scratch_size=64KB so desc-gen overlaps the DMA drain),
landing g4[p, l, 64] = super-row of ids[p, l]. The wanted 16 floats are
selected by weights w[p, l, r] = (ids&3 == r) * (ids != 0), fused with
the mean reduction on DVE:
    out[p, d] = (sum_{l,r} w[p,l,r] * g4[p, l*64+r*16+d]) / max(cnt, 1)

Index tiles (wrapped [16 x num_idxs/16] layout, replicated to 128
partitions) are precomputed host-side from code_ids (pure re-layout of
the input), streamed per tile as an extra int16 input.

Per tile (cost model): Pool 4x(994+2048*0.34) ~= 6.8 us desc-gen, DMA
8192 descs * 256 B ~= 12 us, DVE ~10 us -> DMA-bound ~2.5 ms/core for
196 tiles (vs ~17-28 ms for the 64-instruction-per-tile indirect1d
baseline, which paid 994 ns fixed SWDGE cost per 128 gathered rows).
"""

import numpy as np

PAD_IDX = 0
NUM_CODES = 100000
DIM = 16
B, L = 200000, 64
N_CORES = 8

TILE_ROWS = 128
B_SHARD = 25088                       # 25000 + pad to multiple of 128
N_TILES = B_SHARD // TILE_ROWS        # 196

NSUP = NUM_CODES // 4                 # 25000 super-rows
ELEM = 64                             # f32 per super-row (256 B)
CHUNK = 1024                          # idxs per dma_gather (SWDGE ring cap)
NCHUNK = (TILE_ROWS * L) // CHUNK     # 8
UPT = CHUNK // TILE_ROWS              # 8 l-slots per chunk
GIDX_COLS = NCHUNK * (CHUNK // 16)    # 512 int16 per partition-row per tile
N_QUEUES = 4                          # SWDGE queues to spread chunks over

_PROGRAM_CACHE = {}


def build_program(b_shard=B_SHARD, repeats=1, unroll=14, bufs=2):
    from contextlib import ExitStack

    import concourse.tile as tile
    from concourse import bacc, bass, mybir

    n_tiles = b_shard // TILE_ROWS
    assert b_shard % TILE_ROWS == 0 and n_tiles % unroll == 0

    nc = bacc.Bacc(
        "TRN2",
        target_bir_lowering=False,
        debug=False,
        num_swdge_queues=N_QUEUES,
    )
    ids_t = nc.dram_tensor("code_ids", [b_shard, L], mybir.dt.int32, kind="ExternalInput")
    gidx_t = nc.dram_tensor("gidx", [b_shard, GIDX_COLS], mybir.dt.int16, kind="ExternalInput")
    tbl_t = nc.dram_tensor("emb4", [NSUP, ELEM], mybir.dt.float32, kind="ExternalInput")
    iota_t = nc.dram_tensor("iota4", [128, L * 4], mybir.dt.int32, kind="ExternalInput")
    out_t = nc.dram_tensor("out", [b_shard, DIM], mybir.dt.float32, kind="ExternalOutput")

    with ExitStack() as ctx:
        tc = ctx.enter_context(tile.TileContext(nc))
        const_pool = ctx.enter_context(tc.tile_pool(name="const", bufs=1))
        ids_pool = ctx.enter_context(tc.tile_pool(name="ids", bufs=bufs))
        g_pool = ctx.enter_context(tc.tile_pool(name="g", bufs=bufs))
        s_pool = ctx.enter_context(tc.tile_pool(name="s", bufs=bufs))

        iota4 = const_pool.tile([128, L * 4], mybir.dt.int32, tag="iota4")
        nc.sync.dma_start(out=iota4[:], in_=iota_t[:, :])

        def tile_body(row0_expr):
            ids_tile = ids_pool.tile([128, L], mybir.dt.int32, tag="ids", name=f"ids{nc.next_id()}")
            nc.sync.dma_start(out=ids_tile[:], in_=ids_t[bass.ds(row0_expr, 128), :])
            gidx_tile = ids_pool.tile([128, GIDX_COLS], mybir.dt.int16, tag="gidx", name=f"gi{nc.next_id()}")
            nc.sync.dma_start(out=gidx_tile[:], in_=gidx_t[bass.ds(row0_expr, 128), :])

            g4 = g_pool.tile([128, L * ELEM], mybir.dt.float32, tag="g4", name=f"g{nc.next_id()}")
            for c in range(NCHUNK):
                nc.gpsimd.dma_gather(
                    out_ap=g4[:, c * UPT * ELEM : (c + 1) * UPT * ELEM].rearrange(
                        "p (u e) -> p u e", u=UPT, e=ELEM
                    ),
                    in_ap=tbl_t[:, :],
                    idxs_ap=gidx_tile[:, c * (CHUNK // 16) : (c + 1) * (CHUNK // 16)],
                    num_idxs=CHUNK,
                    num_idxs_reg=CHUNK,
                    elem_size=ELEM,
                    single_packet=False,
                    queue_num=c % N_QUEUES,
                )

            # weights: w[p, l, r] = (ids&3 == r); pad rows excluded via m
            a = s_pool.tile([128, L], mybir.dt.int32, tag="a", name=f"a{nc.next_id()}")
            nc.vector.tensor_scalar(a[:], ids_tile[:], 3, None, op0=mybir.AluOpType.bitwise_and)
            m = s_pool.tile([128, L], mybir.dt.float32, tag="m", name=f"m{nc.next_id()}")
            nc.vector.tensor_scalar(m[:], ids_tile[:], 0, None, op0=mybir.AluOpType.not_equal)
            den = s_pool.tile([128, 1], mybir.dt.float32, tag="den", name=f"d{nc.next_id()}")
            nc.vector.tensor_reduce(den[:], m[:], axis=mybir.AxisListType.X, op=mybir.AluOpType.add)
            nc.vector.tensor_scalar_max(den[:], den[:], 1.0)
            recip = s_pool.tile([128, 1], mybir.dt.float32, tag="recip", name=f"r{nc.next_id()}")
            nc.vector.reciprocal(recip[:], den[:])

            wcmp = s_pool.tile([128, L * 4], mybir.dt.float32, tag="wcmp", name=f"w{nc.next_id()}")
            nc.vector.tensor_tensor(
                out=wcmp[:].rearrange("p (l r) -> p l r", l=L, r=4),
                in0=a[:].broadcast_to([128, L, 4]),
                in1=iota4[:].rearrange("p (l r) -> p l r", l=L, r=4),
                op=mybir.AluOpType.is_equal,
            )
            wm = s_pool.tile([128, L * 4], mybir.dt.float32, tag="wm", name=f"wm{nc.next_id()}")
            nc.vector.tensor_tensor(
                out=wm[:].rearrange("p (l r) -> p l r", l=L, r=4),
                in0=wcmp[:].rearrange("p (l r) -> p l r", l=L, r=4),
                in1=m[:].broadcast_to([128, L, 4]),
                op=mybir.AluOpType.mult,
            )

            P = g_pool.tile([128, L * ELEM], mybir.dt.float32, tag="P", name=f"P{nc.next_id()}")
            nc.vector.tensor_tensor(
                out=P[:].rearrange("p (l r d) -> p l r d", l=L, r=4, d=DIM),
                in0=g4[:].rearrange("p (l r d) -> p l r d", l=L, r=4, d=DIM),
                in1=wm[:].rearrange("p (l r) -> p l r", l=L, r=4).broadcast_to([128, L, 4, DIM]),
                op=mybir.AluOpType.mult,
            )
            acc = s_pool.tile([128, DIM], mybir.dt.float32, tag="acc", name=f"ac{nc.next_id()}")
            nc.vector.tensor_reduce(
                acc[:],
                P[:].rearrange("p (lr d) -> p d lr", lr=L * 4, d=DIM),
                axis=mybir.AxisListType.X,
                op=mybir.AluOpType.add,
            )
            outt = s_pool.tile([128, DIM], mybir.dt.float32, tag="outt", name=f"o{nc.next_id()}")
            nc.vector.tensor_scalar(outt[:], acc[:], recip[:], None, op0=mybir.AluOpType.mult)
            nc.sync.dma_start(out=out_t[bass.ds(row0_expr, 128), :], in_=outt[:])

        if repeats == 1:
            with tc.For_i(0, n_tiles // unroll, 1) as i:
                for u in range(unroll):
                    tile_body(i * (128 * unroll) + u * 128)
        else:
            with tc.For_i(0, repeats, 1) as _r:
                with tc.For_i(0, n_tiles // unroll, 1) as i:
                    for u in range(unroll):
                        tile_body(i * (128 * unroll) + u * 128)

    nc.compile()
    return nc


def _get_program():
    key = (B_SHARD, 1)
    if key not in _PROGRAM_CACHE:
        _PROGRAM_CACHE[key] = build_program()
    return _PROGRAM_CACHE[key]


def _make_gidx(ids_shard: np.ndarray) -> np.ndarray:
    """Wrapped dma_gather index layout for one core shard [b_shard, L].

    Tile t, chunk c, in-chunk j = u*128+p gathers super-row of
    ids[t*128+p, c*UPT+u]; idx j lives at partition j%16 (replicated to
    all 8 16-partition groups), column c*(CHUNK/16) + j//16 of the
    [128, L] int16 per-tile block.
    """
    b_shard = ids_shard.shape[0]
    n_tiles = b_shard // TILE_ROWS
    sup = (ids_shard >> 2).astype(np.int16)          # [b, L]
    s = sup.reshape(n_tiles, TILE_ROWS, NCHUNK, UPT)  # [t, p, c, u]
    s = s.transpose(0, 2, 3, 1)                       # [t, c, u, p]
    s = s.reshape(n_tiles, NCHUNK, CHUNK // 16, 16)   # [t, c, j//16, j%16]
    s = s.transpose(0, 1, 3, 2)                       # [t, c, q, col]
    s = np.tile(s, (1, 1, 8, 1))                      # [t, c, 128, col]
    s = s.transpose(0, 2, 1, 3)                       # [t, 128, c, col]
    return np.ascontiguousarray(s.reshape(b_shard, GIDX_COLS))


def make_in_maps(code_ids: np.ndarray, emb_weight: np.ndarray):
    code_ids = np.ascontiguousarray(np.asarray(code_ids), dtype=np.int32)
    emb_weight = np.ascontiguousarray(np.asarray(emb_weight), dtype=np.float32)
    tbl4 = emb_weight.reshape(NSUP, ELEM)
    b_total = N_CORES * B_SHARD
    ids_pad = np.zeros((b_total, L), dtype=np.int32)
    ids_pad[: code_ids.shape[0], :] = code_ids
    iota4 = np.tile(np.arange(4, dtype=np.int32), (128, L)).reshape(128, L * 4)
    maps = []
    for i in range(N_CORES):
        shard = ids_pad[i * B_SHARD : (i + 1) * B_SHARD]
        maps.append(
            {
                "code_ids": shard,
                "gidx": _make_gidx(shard),
                "emb4": tbl4,
                "iota4": iota4,
            }
        )
    return maps


def kernel(code_ids: np.ndarray, emb_weight: np.ndarray, **kwargs) -> np.ndarray:
    from concourse import bass_utils

    nc = _get_program()
    in_maps = make_in_maps(code_ids, emb_weight)
    res = bass_utils.run_bass_kernel_spmd(nc, in_maps, core_ids=list(range(N_CORES)))
    out = np.concatenate([res.results[i]["out"] for i in range(N_CORES)], axis=0)
    return out[: np.asarray(code_ids).shape[0]]


if __name__ == "__main__":
    rng = np.random.default_rng(0)
    ids = rng.integers(0, NUM_CODES, size=(B, L)).astype(np.int32)
    w = rng.standard_normal((NUM_CODES, DIM)).astype(np.float32)
    o = kernel(code_ids=ids, emb_weight=w)
    print(o.shape, o.dtype, o[:2, :4])
